# revision 2
# baseline (speedup 1.0000x reference)
"""LocalAttention2d Bass kernel for 8 Trainium2 NeuronCores.

Strategy: pure data parallel over batch (8 batches/core).  The module only
attends over an 8x8 window of data-dependent spatial positions per batch, so
instead of streaming all of q (64 MB/core) the kernel computes the window
indices on-device and gathers just the 64 needed feature rows per batch
(512 B each) with an indirect DMA from a host-pretransposed [B*H*W, D] table.

Host-side work is limited to data-INdependent layout prep (transposes of q /
c_t / W_p, constant iota tables); every data-dependent step (p_t, rounding,
window indices, shift, softmax, output) runs on the NeuronCore.
"""

import sys

import numpy as np

try:
    import concourse.bass_utils as _bu
except ImportError:  # fresh grading dir: fall back to the repo checkout
    sys.path.insert(0, "/opt/trn_rl_repo")
    import concourse.bass_utils as _bu

import concourse.bacc as bacc
import concourse.bass as bass
import concourse.mybir as mybir
import concourse.tile as tile
from concourse.bass import IndirectOffsetOnAxis

B, D, H, W = 64, 128, 128, 128
CSZ = 256
R = 8                     # window rows == cols
NCORES = 8
BPC = B // NCORES         # batches per core
HW = H * W
ROWS = H + 1              # 129, padded row count in the reference
NW = R * R                # 64 window positions
F32 = mybir.dt.float32
I32 = mybir.dt.int32

AOP = mybir.AluOpType
ACT = mybir.ActivationFunctionType


# packed-constant column layout in aux128 [128, 404]:
#   0:128 identity | 128:256 wa0 | 256:384 wa1 | 384:392 ct0 | 392:400 ct1
#   | 400:402 wp0 | 402:404 wp1
# aux8 [8, 521]:  0:512 dmask | 512:520 offs | 520:521 boff
AUX128_W = 404
AUX8_W = 521


def _build(stage="full"):
    nc = bacc.Bacc(
        "TRN2",
        target_bir_lowering=False,
        debug=False,
        num_devices=NCORES,
    )

    def _stop(sp, nc, ap, out):
        """debug: dump (part of) an intermediate into `out` and finish"""
        o = sp.tile([BPC, D], F32)
        nc.gpsimd.memset(o[:], 0.0)
        p = min(ap.shape[0], BPC)
        f = min(ap.shape[1], D)
        nc.vector.tensor_copy(o[0:p, 0:f], ap[0:p, 0:f])
        nc.sync.dma_start(out=out[:], in_=o[:])
        return True

    qhw = nc.dram_tensor("qhw", [BPC * HW, D], F32, kind="ExternalInput")
    aux128 = nc.dram_tensor("aux128", [128, AUX128_W], F32, kind="ExternalInput")
    aux8 = nc.dram_tensor("aux8", [BPC, AUX8_W], F32, kind="ExternalInput")
    out = nc.dram_tensor("out", [BPC, D], F32, kind="ExternalOutput")

    with tile.TileContext(nc) as tc:
        with (
            tc.tile_pool(name="sb", bufs=1) as sp,
            tc.tile_pool(name="ps", bufs=1, space="PSUM") as pp,
        ):
            # ---- load packed operands (one DMA sem tick each) ------------
            a128 = sp.tile([128, AUX128_W], F32)
            nc.sync.dma_start(out=a128[:], in_=aux128[:])
            a8 = sp.tile([BPC, AUX8_W], F32)
            nc.sync.dma_start(out=a8[:], in_=aux8[:])

            ident = a128[:, 0:128]
            wa0 = a128[:, 128:256]
            wa1 = a128[:, 256:384]
            ct0 = a128[:, 384:392]
            ct1 = a128[:, 392:400]
            wp0 = a128[:, 400:402]
            wp1 = a128[:, 402:404]
            dmask_s = a8[:, 0:512]
            offs_s = a8[:, 512:520]
            boff_s = a8[:, 520:521]

            # ---- p_t = 128*sigmoid(c_t @ W_p.T)  -> [BPC, 2] -------------
            pt_p = pp.tile([BPC, 2], F32)
            nc.tensor.matmul(out=pt_p[:], lhsT=ct0, rhs=wp0, start=True, stop=False)
            nc.tensor.matmul(out=pt_p[:], lhsT=ct1, rhs=wp1, start=False, stop=True)
            pt128 = sp.tile([BPC, 2], F32)
            nc.scalar.activation(out=pt128[:], in_=pt_p[:], func=ACT.Sigmoid)
            nc.vector.tensor_scalar_mul(pt128[:], pt128[:], float(H))

            # vT[d, b] = sum_c W_a[c, d] * c_t[b, c]   -> [D, BPC]
            vT_p = pp.tile([D, BPC], F32)
            nc.tensor.matmul(out=vT_p[:], lhsT=wa0, rhs=ct0, start=True, stop=False)
            nc.tensor.matmul(out=vT_p[:], lhsT=wa1, rhs=ct1, start=False, stop=True)
            vT_s = sp.tile([D, BPC], F32)
            nc.vector.tensor_copy(vT_s[:], vT_p[:])

            # ---- round to ints (round-to-nearest via f32->i32 convert) ---
            pri = sp.tile([BPC, 2], I32)
            nc.vector.tensor_copy(pri[:], pt128[:])
            prf = sp.tile([BPC, 2], F32)
            nc.vector.tensor_copy(prf[:], pri[:])

            # ---- window indices r/c: mod(clip(p_round + offs, 0, 129), 129)
            rr = sp.tile([BPC, R], F32)
            cc = sp.tile([BPC, R], F32)
            nc.vector.tensor_tensor(
                out=rr[:], in0=prf[:, 0:1].to_broadcast([BPC, R]), in1=offs_s, op=AOP.add
            )
            nc.vector.tensor_tensor(
                out=cc[:], in0=prf[:, 1:2].to_broadcast([BPC, R]), in1=offs_s, op=AOP.add
            )
            for t in (rr, cc):
                nc.vector.tensor_scalar(
                    out=t[:], in0=t[:], scalar1=0.0, scalar2=float(ROWS),
                    op0=AOP.max, op1=AOP.min,
                )
            rmask = sp.tile([BPC, R], F32)
            cmask = sp.tile([BPC, R], F32)
            nc.vector.tensor_scalar(out=rmask[:], in0=rr[:], scalar1=float(ROWS), scalar2=None, op0=AOP.is_lt)
            nc.vector.tensor_scalar(out=cmask[:], in0=cc[:], scalar1=float(ROWS), scalar2=None, op0=AOP.is_lt)
            nc.vector.tensor_tensor(out=rr[:], in0=rr[:], in1=rmask[:], op=AOP.mult)
            nc.vector.tensor_tensor(out=cc[:], in0=cc[:], in1=cmask[:], op=AOP.mult)

            # valid: r > 0, c > 0
            rpos = sp.tile([BPC, R], F32)
            cpos = sp.tile([BPC, R], F32)
            nc.vector.tensor_scalar(out=rpos[:], in0=rr[:], scalar1=0.0, scalar2=None, op0=AOP.is_gt)
            nc.vector.tensor_scalar(out=cpos[:], in0=cc[:], scalar1=0.0, scalar2=None, op0=AOP.is_gt)

            # rm1 = max(r-1, 0), cm1 = max(c-1, 0)
            rm1 = sp.tile([BPC, R], F32)
            cm1 = sp.tile([BPC, R], F32)
            nc.vector.tensor_scalar(out=rm1[:], in0=rr[:], scalar1=1.0, scalar2=0.0, op0=AOP.subtract, op1=AOP.max)
            nc.vector.tensor_scalar(out=cm1[:], in0=cc[:], scalar1=1.0, scalar2=0.0, op0=AOP.subtract, op1=AOP.max)

            # rexp = (rm1 - p_t0)^2 / 8 ; cexp likewise
            rexp = sp.tile([BPC, R], F32)
            cexp = sp.tile([BPC, R], F32)
            nc.vector.tensor_tensor(out=rexp[:], in0=rm1[:], in1=pt128[:, 0:1].to_broadcast([BPC, R]), op=AOP.subtract)
            nc.vector.tensor_tensor(out=cexp[:], in0=cm1[:], in1=pt128[:, 1:2].to_broadcast([BPC, R]), op=AOP.subtract)
            nc.vector.tensor_tensor(out=rexp[:], in0=rexp[:], in1=rexp[:], op=AOP.mult)
            nc.vector.tensor_tensor(out=cexp[:], in0=cexp[:], in1=cexp[:], op=AOP.mult)
            nc.vector.tensor_scalar_mul(rexp[:], rexp[:], 0.125)
            nc.vector.tensor_scalar_mul(cexp[:], cexp[:], 0.125)

            # ---- expand to [BPC, 64]: index i*8+j  (i from rows, j cols) --
            def imaj(t):  # value at (b, i) repeated over j
                return t[:].unsqueeze(2).to_broadcast([BPC, R, R])

            def jmin(t):  # value at (b, j) cycling fastest
                return t[:].unsqueeze(1).to_broadcast([BPC, R, R])

            shift64 = sp.tile([BPC, NW], F32)
            nc.vector.tensor_tensor(
                out=shift64[:].rearrange("p (i j) -> p i j", i=R),
                in0=imaj(rexp), in1=jmin(cexp), op=AOP.add,
            )
            valid64 = sp.tile([BPC, NW], F32)
            nc.vector.tensor_tensor(
                out=valid64[:].rearrange("p (i j) -> p i j", i=R),
                in0=imaj(rpos), in1=jmin(cpos), op=AOP.mult,
            )
            rm1x = sp.tile([BPC, R], F32)
            nc.vector.tensor_scalar_mul(rm1x[:], rm1[:], float(W))
            gidx64 = sp.tile([BPC, NW], F32)
            nc.vector.tensor_tensor(
                out=gidx64[:].rearrange("p (i j) -> p i j", i=R),
                in0=imaj(rm1x), in1=jmin(cm1), op=AOP.add,
            )
            nc.vector.tensor_scalar(
                out=gidx64[:], in0=gidx64[:], scalar1=boff_s, scalar2=None, op0=AOP.add
            )

            # ---- transpose indices into per-partition layout -------------
            gT_p = pp.tile([NW, BPC], F32)
            nc.tensor.transpose(gT_p[:], gidx64[:], ident[0:BPC, 0:BPC])
            gT_s = sp.tile([NW, BPC], F32)
            nc.vector.tensor_copy(gT_s[:], gT_p[:])

            ntiles = (BPC * NW) // 128  # 4 tiles of 128 gathered rows
            idx_tiles = []
            qg_tiles = []
            for t in range(ntiles):
                idx_t = sp.tile([128, 1], I32, tag=f"idx{t}")
                nc.vector.tensor_copy(idx_t[0:NW, :], gT_s[:, 2 * t : 2 * t + 1])
                nc.vector.tensor_copy(idx_t[NW:128, :], gT_s[:, 2 * t + 1 : 2 * t + 2])
                idx_tiles.append(idx_t)

            # ---- the sparse gather: 128 rows x 512B per tile -------------
            for t in range(ntiles):
                qg_t = sp.tile([128, D], F32, tag=f"qg{t}")
                nc.gpsimd.indirect_dma_start(
                    out=qg_t[:],
                    out_offset=None,
                    in_=qhw[:],
                    in_offset=IndirectOffsetOnAxis(ap=idx_tiles[t][:, :1], axis=0),
                )
                qg_tiles.append(qg_t)

            # ---- transpose gathered rows to [d, window] ------------------
            qgT_s = sp.tile([D, BPC * NW], F32)
            for t in range(ntiles):
                tr_p = pp.tile([D, 128], F32, tag="tr")
                nc.tensor.transpose(tr_p[:], qg_tiles[t][:], ident)
                nc.vector.tensor_copy(qgT_s[:, t * 128 : (t + 1) * 128], tr_p[:])

            # ---- scores: [BPC, BPC*NW] one matmul ------------------------
            # Row b contains scores of ALL 512 gathered rows against v_b;
            # only the diagonal block [b, b*64:(b+1)*64] is real. Off-diag
            # and invalid positions are pushed to -1e30 so their softmax
            # weight is exactly 0, making the final accumulation correct.
            scores_p = pp.tile([BPC, BPC * NW], F32)
            for t in range(ntiles):
                nc.tensor.matmul(
                    out=scores_p[:, t * 128 : (t + 1) * 128],
                    lhsT=vT_s[:],
                    rhs=qgT_s[:, t * 128 : (t + 1) * 128],
                    start=True, stop=True,
                )

            def tiled64(t):  # [BPC, 64] -> [BPC, 8, 64] repeating along dim 1
                return t[:].unsqueeze(1).to_broadcast([BPC, BPC, NW])

            def v3(t):  # [BPC, 512] viewed as [BPC, 8, 64]
                return t[:].rearrange("p (m k) -> p m k", m=BPC)

            v512 = sp.tile([BPC, BPC * NW], F32)
            nc.vector.tensor_tensor(out=v3(v512), in0=tiled64(valid64), in1=dmask_s.rearrange("p (m k) -> p m k", m=BPC), op=AOP.mult)
            am = sp.tile([BPC, BPC * NW], F32)
            nc.vector.tensor_tensor(out=v3(am), in0=scores_p[:].rearrange("p (m k) -> p m k", m=BPC), in1=tiled64(shift64), op=AOP.subtract)
            negbig = sp.tile([BPC, BPC * NW], F32)
            nc.vector.tensor_scalar(
                out=negbig[:], in0=v512[:], scalar1=1.0, scalar2=1e30,
                op0=AOP.subtract, op1=AOP.mult,
            )
            nc.vector.tensor_tensor(out=am[:], in0=am[:], in1=negbig[:], op=AOP.add)

            # ---- softmax (exp + row-sum fused on ACT) --------------------
            e_s = sp.tile([BPC, BPC * NW], F32)
            ssum = sp.tile([BPC, 1], F32)
            nc.scalar.activation(out=e_s[:], in_=am[:], func=ACT.Exp, accum_out=ssum[:])
            sinv = sp.tile([BPC, 1], F32)
            nc.vector.reciprocal(sinv[:], ssum[:])
            w512 = sp.tile([BPC, BPC * NW], F32)
            nc.vector.tensor_tensor(
                out=w512[:], in0=e_s[:], in1=sinv[:, 0:1].to_broadcast([BPC, BPC * NW]), op=AOP.mult
            )

            # ---- weighted sum: outT[d, b] = sum_pos qg[pos, d] wT[pos, b]
            outT_p = pp.tile([D, BPC], F32)
            for t in range(ntiles):
                wT_p = pp.tile([128, BPC], F32, tag="wt")
                nc.tensor.transpose(wT_p[:], w512[:, t * 128 : (t + 1) * 128], ident[0:BPC, 0:BPC])
                wT_s = sp.tile([128, BPC], F32, tag="wts")
                nc.vector.tensor_copy(wT_s[:], wT_p[:])
                nc.tensor.matmul(
                    out=outT_p[:],
                    lhsT=qg_tiles[t][:],
                    rhs=wT_s[:],
                    start=(t == 0), stop=(t == ntiles - 1),
                )
            outT_s = sp.tile([D, BPC], F32)
            nc.vector.tensor_copy(outT_s[:], outT_p[:])
            outf_p = pp.tile([BPC, D], F32)
            nc.tensor.transpose(outf_p[:], outT_s[:], ident)
            outf_s = sp.tile([BPC, D], F32)
            nc.vector.tensor_copy(outf_s[:], outf_p[:])
            nc.sync.dma_start(out=out[:], in_=outf_s[:])

    nc.compile()
    return nc


_CACHE = {}


def _prep_in_maps(q, c_t, W_a, W_p):
    waT2 = W_a.astype(np.float32).reshape(2, 128, D)      # [2, 128, 128] row blocks
    wpT2 = W_p.T.astype(np.float32).reshape(2, 128, 2)    # [2, 128, 2] row blocks

    aux8_np = np.zeros((BPC, AUX8_W), np.float32)
    aux8_np[:, 0:512] = np.kron(np.eye(BPC, dtype=np.float32), np.ones((1, NW), np.float32))
    aux8_np[:, 512:520] = np.tile((np.arange(R) - (R // 2 - 1)).astype(np.float32), (BPC, 1))
    aux8_np[:, 520] = np.arange(BPC, dtype=np.float32) * HW

    base128 = np.zeros((128, AUX128_W), np.float32)
    base128[:, 0:128] = np.eye(128, dtype=np.float32)
    base128[:, 128:256] = waT2[0]
    base128[:, 256:384] = waT2[1]
    base128[:, 400:402] = wpT2[0]
    base128[:, 402:404] = wpT2[1]

    in_maps = []
    for c in range(NCORES):
        qs = q[c * BPC : (c + 1) * BPC]  # [BPC, D, H, W]
        qhw_np = np.ascontiguousarray(qs.transpose(0, 2, 3, 1)).reshape(BPC * HW, D)
        ctT_np = np.ascontiguousarray(c_t[c * BPC : (c + 1) * BPC].T)  # [CSZ, BPC]
        aux128_np = base128.copy()
        aux128_np[:, 384:392] = ctT_np[0:128]
        aux128_np[:, 392:400] = ctT_np[128:256]
        in_maps.append({"qhw": qhw_np, "aux128": aux128_np, "aux8": aux8_np})
    return in_maps


def run(trace=False, tmpdir=None, **inputs):
    q = np.asarray(inputs["q"], dtype=np.float32)
    c_t = np.asarray(inputs["c_t"], dtype=np.float32)
    W_a = np.asarray(inputs["W_a"], dtype=np.float32)
    W_p = np.asarray(inputs["W_p"], dtype=np.float32)
    if "nc" not in _CACHE:
        _CACHE["nc"] = _build()
    in_maps = _prep_in_maps(q, c_t, W_a, W_p)
    res = _bu.run_bass_kernel_spmd(
        _CACHE["nc"], in_maps, core_ids=list(range(NCORES)), trace=trace,
        tmpdir=tmpdir,
    )
    outp = np.concatenate([r["out"] for r in res.results], axis=0)
    return outp, res


def kernel(**inputs):
    outp, _ = run(trace=False, **inputs)
    return outp



# revision 5
# speedup vs baseline: 1.1606x; 1.1606x over previous
"""LocalAttention2d Bass kernel for 8 Trainium2 NeuronCores.

Strategy: pure data parallel over batch (8 batches/core).  The module only
attends over an 8x8 window of data-dependent spatial positions per batch, so
instead of streaming all of q (64 MB/core) the kernel computes the window
indices on-device and gathers just the 64 needed feature rows per batch
(512 B each) with an indirect DMA from a host-pretransposed [B*H*W, D] table.

Layout: all data-dependent math runs "window-transposed" — window position on
the partition axis (r-part in partitions 0:64, c-part in 64:128) and batch on
the free axis — so every vector op uses all 128 lanes, the window indices are
born in the partition-major layout the indirect DMA wants (no transpose), and
softmax runs unnormalized with a ones-column accumulating the denominator in
the same PSUM matmul as the weighted sum (no weight/output transposes).

Host-side work is limited to data-INdependent layout prep (transposes of q /
c_t / W_p, constant tables); every data-dependent step (p_t, rounding,
window indices, shift, softmax, output) runs on the NeuronCore.
"""

import sys

import numpy as np

try:
    import concourse.bass_utils as _bu
except ImportError:  # fresh grading dir: fall back to the repo checkout
    sys.path.insert(0, "/opt/trn_rl_repo")
    import concourse.bass_utils as _bu

import concourse.bacc as bacc
import concourse.bass as bass
import concourse.mybir as mybir
import concourse.tile as tile
from concourse.bass import IndirectOffsetOnAxis

B, D, H, W = 64, 128, 128, 128
CSZ = 256
R = 8                     # window rows == cols
NCORES = 8
BPC = B // NCORES         # batches per core
HW = H * W
ROWS = H + 1              # 129, padded row count in the reference
NW = R * R                # 64 window positions
NT = (BPC * NW) // 128    # 4 gather tiles of 128 rows (2 batches each)
QGS = 132                 # gather tile stride: 128 data cols + ones col + pad
F32 = mybir.dt.float32
I32 = mybir.dt.int32

AOP = mybir.AluOpType
ACT = mybir.ActivationFunctionType

# auxS [128, 153] column layout (critical-path constants, first DMA):
#   0:8 ct0 | 8:16 ct1 | 16:18 wp0 | 18:20 wp1 | 20:148 selc (parts 0:2)
#   | 148:149 oi | 149:153 unused | 153:161 boffc
AUXS_W = 161
# auxL [128, 384]: 0:128 wa0 | 128:256 wa1 | 256:384 identity
AUXL_W = 384


def _build():
    nc = bacc.Bacc(
        "TRN2",
        target_bir_lowering=False,
        debug=False,
        num_devices=NCORES,
    )

    qhw = nc.dram_tensor("qhw", [BPC * HW, D], F32, kind="ExternalInput")
    auxS = nc.dram_tensor("auxS", [128, AUXS_W], F32, kind="ExternalInput")
    auxL = nc.dram_tensor("auxL", [128, AUXL_W], F32, kind="ExternalInput")
    out = nc.dram_tensor("out", [BPC, D], F32, kind="ExternalOutput")

    with tile.TileContext(nc) as tc:
        with (
            tc.tile_pool(name="sb", bufs=1) as sp,
            tc.tile_pool(name="ps", bufs=1, space="PSUM") as pp,
        ):
            # ---- input DMAs: small critical constants first ---------------
            aS = sp.tile([128, AUXS_W], F32)
            nc.sync.dma_start(out=aS[:], in_=auxS[:])
            aL = sp.tile([128, AUXL_W], F32)
            nc.sync.dma_start(out=aL[:], in_=auxL[:])

            ct0 = aS[:, 0:8]
            ct1 = aS[:, 8:16]
            wp0 = aS[:, 16:18]
            wp1 = aS[:, 18:20]
            selc = aS[0:2, 20:148]
            oi = aS[:, 148:149]
            boffc = aS[0:64, 153:161]   # [64, 8], col b = b*HW
            wa0 = aL[:, 0:128]
            wa1 = aL[:, 128:256]
            ident = aL[:, 256:384]

            # ---- early memsets (no data deps) -----------------------------
            qgall = sp.tile([128, NT * QGS], F32)   # gather tiles + ones cols
            nc.gpsimd.memset(qgall[:], 1.0)
            addfull = sp.tile([128, NT * BPC], F32)  # -shift/-inf mask, [128,32]
            nc.gpsimd.memset(addfull[:], -1e30)

            # ---- p_t: ptT[2,8] = (W_p c_t)^T, broadcast 128*sigmoid -------
            ptT_ps = pp.tile([2, BPC], F32)
            nc.tensor.matmul(out=ptT_ps[:], lhsT=wp0, rhs=ct0, start=True, stop=False)
            nc.tensor.matmul(out=ptT_ps[:], lhsT=wp1, rhs=ct1, start=False, stop=True)
            sig8 = sp.tile([2, BPC], F32)
            nc.scalar.activation(out=sig8[:], in_=ptT_ps[:], func=ACT.Sigmoid)
            # selc rows carry 128.0 -> ptb = 128*sigmoid; row part in
            # partitions 0:64 (p_t[:,0]), col part in partitions 64:128
            ptb_ps = pp.tile([128, BPC], F32)
            nc.tensor.matmul(out=ptb_ps[:], lhsT=selc, rhs=sig8[:], start=True, stop=True)

            # ---- round via f32->i32->f32 cast -----------------------------
            pri = sp.tile([128, BPC], I32)
            nc.vector.tensor_copy(pri[:], ptb_ps[:])
            prf = sp.tile([128, BPC], F32)
            nc.vector.tensor_copy(prf[:], pri[:])

            # ---- window indices: a=max(p+o,0); rr=min(a,129) mod 129 ------
            aa = sp.tile([128, BPC], F32)
            nc.vector.tensor_scalar(
                out=aa[:], in0=prf[:], scalar1=oi, scalar2=0.0,
                op0=AOP.add, op1=AOP.max,
            )
            amask = sp.tile([128, BPC], F32)
            nc.vector.tensor_scalar(
                out=amask[:], in0=aa[:], scalar1=float(ROWS), scalar2=None, op0=AOP.is_lt
            )
            rr = sp.tile([128, BPC], F32)
            nc.vector.tensor_tensor(out=rr[:], in0=aa[:], in1=amask[:], op=AOP.mult)
            # rm1 = max(rr-1, 0)
            rm1 = sp.tile([128, BPC], F32)
            nc.vector.tensor_scalar(
                out=rm1[:], in0=rr[:], scalar1=1.0, scalar2=0.0,
                op0=AOP.subtract, op1=AOP.max,
            )
            # flat row index gidx = rm1_r*W + rm1_c + b*HW  (in [64, 8])
            cm = sp.tile([64, BPC], F32)
            nc.vector.tensor_copy(cm[:], rm1[64:128, :])       # c-half to base 0
            rm1x = sp.tile([64, BPC], F32)
            nc.vector.tensor_scalar_mul(rm1x[:], rm1[0:64, :], float(W))
            gidx = sp.tile([64, BPC], F32)
            nc.vector.tensor_tensor(out=gidx[:], in0=rm1x[:], in1=cm[:], op=AOP.add)
            gb = sp.tile([64, BPC], F32)
            nc.vector.tensor_tensor(out=gb[:], in0=gidx[:], in1=boffc, op=AOP.add)
            # interleave batch pairs into gather-tile columns [128, 4]
            idxf = sp.tile([128, NT], F32)
            nc.vector.tensor_copy(idxf[0:64, :], gb[:, 0:8:2])
            nc.vector.tensor_copy(idxf[64:128, :], gb[:, 1:8:2])
            idx128 = sp.tile([128, NT], I32)
            nc.vector.tensor_copy(idx128[:], idxf[:])

            # ---- sparse gather: 4 x 128 rows x 512B -----------------------
            for t in range(NT):
                nc.gpsimd.indirect_dma_start(
                    out=qgall[:, t * QGS : t * QGS + D],
                    out_offset=None,
                    in_=qhw[:],
                    in_offset=IndirectOffsetOnAxis(ap=idx128[:, t : t + 1], axis=0),
                )

            # ---- shift/valid mask (overlaps the gather) -------------------
            rexpd = sp.tile([128, BPC], F32)
            nc.vector.tensor_tensor(out=rexpd[:], in0=rm1[:], in1=ptb_ps[:], op=AOP.subtract)
            rexp = sp.tile([128, BPC], F32)
            # Square(x/sqrt(8)) = x^2/8
            nc.scalar.activation(out=rexp[:], in_=rexpd[:], func=ACT.Square,
                                 scale=0.35355339059327373)
            rpos = sp.tile([128, BPC], F32)
            nc.vector.tensor_scalar(
                out=rpos[:], in0=rr[:], scalar1=0.0, scalar2=None, op0=AOP.is_gt
            )
            rexpc = sp.tile([64, BPC], F32)
            nc.vector.tensor_copy(rexpc[:], rexp[64:128, :])
            shiftT = sp.tile([64, BPC], F32)
            nc.vector.tensor_tensor(out=shiftT[:], in0=rexp[0:64, :], in1=rexpc[:], op=AOP.add)
            cposc = sp.tile([64, BPC], F32)
            nc.vector.tensor_copy(cposc[:], rpos[64:128, :])
            validT = sp.tile([64, BPC], F32)
            nc.vector.tensor_tensor(out=validT[:], in0=rpos[0:64, :], in1=cposc[:], op=AOP.mult)
            negT = sp.tile([64, BPC], F32)
            nc.vector.tensor_scalar(
                out=negT[:], in0=validT[:], scalar1=1.0, scalar2=1e30,
                op0=AOP.subtract, op1=AOP.mult,
            )
            addT = sp.tile([64, BPC], F32)
            nc.vector.tensor_tensor(out=addT[:], in0=negT[:], in1=shiftT[:], op=AOP.subtract)
            # place per-batch columns into the block layout: col 8t+b is the
            # score of tile t's rows against batch b; real pairs are
            # (t, 2t) upper half and (t, 2t+1) lower half -> cols 10t, 10t+1
            nc.vector.tensor_copy(addfull[0:64, 0:31:10], addT[:, 0:8:2])
            nc.vector.tensor_copy(addfull[64:128, 1:32:10], addT[:, 1:8:2])

            # ---- vT[d,b] = sum_c W_a[c,d] c_t[c,b] ------------------------
            vT_ps = pp.tile([D, BPC], F32)
            nc.tensor.matmul(out=vT_ps[:], lhsT=wa0, rhs=ct0, start=True, stop=False)
            nc.tensor.matmul(out=vT_ps[:], lhsT=wa1, rhs=ct1, start=False, stop=True)
            vT_sb = sp.tile([D, BPC], F32)
            nc.vector.tensor_copy(vT_sb[:], vT_ps[:])

            # ---- transpose gathered rows, scores in [128k, 8b] strips -----
            qgT_sb = sp.tile([D, NT * 128], F32)
            scores_ps = pp.tile([128, NT * BPC], F32)
            for t in range(NT):
                tr_ps = pp.tile([D, 128], F32, tag=f"tr{t % 2}")
                nc.tensor.transpose(tr_ps[:], qgall[:, t * QGS : t * QGS + D], ident)
                nc.vector.tensor_copy(qgT_sb[:, t * 128 : (t + 1) * 128], tr_ps[:])
                nc.tensor.matmul(
                    out=scores_ps[:, t * BPC : (t + 1) * BPC],
                    lhsT=qgT_sb[:, t * 128 : (t + 1) * 128],
                    rhs=vT_sb[:],
                    start=True, stop=True,
                )

            # ---- masked, shifted, exp'd scores ----------------------------
            am = sp.tile([128, NT * BPC], F32)
            nc.vector.tensor_tensor(out=am[:], in0=scores_ps[:], in1=addfull[:], op=AOP.add)
            e_sb = sp.tile([128, NT * BPC], F32)
            nc.scalar.activation(out=e_sb[:], in_=am[:], func=ACT.Exp)

            # ---- unnormalized weighted sum + denominator in one PSUM ------
            # out_ps[b, 0:128] = sum_k e[k,b] qg[k,:], out_ps[b,128] = sum_k e[k,b]
            out_ps = pp.tile([BPC, D + 1], F32)
            for t in range(NT):
                nc.tensor.matmul(
                    out=out_ps[:],
                    lhsT=e_sb[:, t * BPC : (t + 1) * BPC],
                    rhs=qgall[:, t * QGS : t * QGS + D + 1],
                    start=(t == 0), stop=(t == NT - 1),
                )
            sinv = sp.tile([BPC, 1], F32)
            nc.vector.reciprocal(sinv[:], out_ps[:, D : D + 1])
            outf = sp.tile([BPC, D], F32)
            nc.vector.tensor_scalar(
                out=outf[:], in0=out_ps[:, 0:D], scalar1=sinv[:, 0:1], scalar2=None,
                op0=AOP.mult,
            )
            nc.sync.dma_start(out=out[:], in_=outf[:])

    nc.compile()
    return nc


_CACHE = {}


def _prep_in_maps(q, c_t, W_a, W_p):
    offs = (np.arange(R) - (R // 2 - 1)).astype(np.float32)  # [-3..4]
    p = np.arange(128)
    oi_np = np.where(p < 64, offs[(p // 8) % 8], offs[p % 8]).astype(np.float32)

    selc_np = np.zeros((2, 128), np.float32)
    selc_np[0, 0:64] = float(H)
    selc_np[1, 64:128] = float(H)

    auxS_base = np.zeros((128, AUXS_W), np.float32)
    auxS_base[:, 16:18] = W_p.T.astype(np.float32)[0:128]
    auxS_base[:, 18:20] = W_p.T.astype(np.float32)[128:256]
    auxS_base[0:2, 20:148] = selc_np
    auxS_base[:, 148] = oi_np
    auxS_base[0:64, 153:161] = (np.arange(BPC, dtype=np.float32) * HW)[None, :]

    auxL_np = np.zeros((128, AUXL_W), np.float32)
    auxL_np[:, 0:128] = W_a.astype(np.float32)[0:128]
    auxL_np[:, 128:256] = W_a.astype(np.float32)[128:256]
    auxL_np[:, 256:384] = np.eye(128, dtype=np.float32)

    in_maps = []
    for c in range(NCORES):
        qs = q[c * BPC : (c + 1) * BPC]  # [BPC, D, H, W]
        qhw_np = np.ascontiguousarray(qs.transpose(0, 2, 3, 1)).reshape(BPC * HW, D)
        ctT_np = np.ascontiguousarray(c_t[c * BPC : (c + 1) * BPC].T)  # [CSZ, BPC]
        auxS_np = auxS_base.copy()
        auxS_np[:, 0:8] = ctT_np[0:128]
        auxS_np[:, 8:16] = ctT_np[128:256]
        in_maps.append({"qhw": qhw_np, "auxS": auxS_np, "auxL": auxL_np})
    return in_maps


def run(trace=False, tmpdir=None, **inputs):
    q = np.asarray(inputs["q"], dtype=np.float32)
    c_t = np.asarray(inputs["c_t"], dtype=np.float32)
    W_a = np.asarray(inputs["W_a"], dtype=np.float32)
    W_p = np.asarray(inputs["W_p"], dtype=np.float32)
    if "nc" not in _CACHE:
        _CACHE["nc"] = _build()
    in_maps = _prep_in_maps(q, c_t, W_a, W_p)
    res = _bu.run_bass_kernel_spmd(
        _CACHE["nc"], in_maps, core_ids=list(range(NCORES)), trace=trace,
        tmpdir=tmpdir,
    )
    outp = np.concatenate([r["out"] for r in res.results], axis=0)
    return outp, res


def kernel(**inputs):
    outp, _ = run(trace=False, **inputs)
    return outp


# revision 15
# speedup vs baseline: 1.1817x; 1.0182x over previous
"""LocalAttention2d Bass kernel for 8 Trainium2 NeuronCores.

Strategy: pure data parallel over batch (8 batches/core).  The module only
attends over an 8x8 window of data-dependent spatial positions per batch, so
instead of streaming all of q (64 MB/core) the kernel computes the window
indices on-device and gathers just the 64 needed feature rows per batch
(512 B each) with an indirect DMA from a host-pretransposed [B*H*W, D] table.

Layout: all data-dependent math runs "window-transposed" — window position on
the partition axis (r-part in partitions 0:64, c-part in 64:128) and batch on
the free axis — so every vector op uses all 128 lanes, the window indices are
born in the partition-major layout the indirect DMA wants (no transpose), and
softmax runs unnormalized with a ones-column accumulating the denominator in
the same PSUM matmul as the weighted sum (no weight/output transposes).

The Gaussian shift and validity mask are folded into the scores PSUM before
the score matmuls via two constant-matrix matmuls (a banded half-combiner and
a rank-1 column-constant), using the linear mask -BIG*(2 - vr - vc) instead
of the product (vr & vc), so no cross-partition vector work is needed and the
exp() activation reads PSUM directly.

Host-side work is limited to data-INdependent layout prep (transposes of q /
c_t / W_p, constant tables); every data-dependent step (p_t, rounding,
window indices, shift, softmax, output) runs on the NeuronCore.
"""

import sys

import numpy as np

try:
    import concourse.bass_utils as _bu
except ImportError:  # fresh grading dir: fall back to the repo checkout
    sys.path.insert(0, "/opt/trn_rl_repo")
    import concourse.bass_utils as _bu

import concourse.bacc as bacc
import concourse.bass as bass
import concourse.mybir as mybir
import concourse.tile as tile
from concourse.bass import IndirectOffsetOnAxis

B, D, H, W = 64, 128, 128, 128
CSZ = 256
R = 8                     # window rows == cols
NCORES = 8
BPC = B // NCORES         # batches per core
HW = H * W
ROWS = H + 1              # 129, padded row count in the reference
NW = R * R                # 64 window positions
NT = (BPC * NW) // 128    # 4 gather tiles of 128 rows (2 batches each)
QGS = 132                 # gather tile stride: 128 data cols + ones col + pad
F32 = mybir.dt.float32
F32R = mybir.dt.float32r
I32 = mybir.dt.int32

AOP = mybir.AluOpType
ACT = mybir.ActivationFunctionType

GATHER_SINGLE = False      # one indirect DMA with a [128, NT] offset AP

# auxS [128, 161] column layout (critical-path constants, first DMA):
#   0:8 ct0 | 8:16 ct1 | 16:18 wp0 | 18:20 wp1 | 20:148 selc (parts 0:2)
#   | 148:149 oi | 153:161 boffc (parts 0:64)
AUXS_W = 161
# auxL [128, 672]: 0:128 wa0 | 128:256 wa1 | 256:384 identity
#   | 384:512 band1 | 512:640 e0sel | 640:672 constm (part 0)
AUXL_W = 672


def _build():
    nc = bacc.Bacc(
        "TRN2",
        target_bir_lowering=False,
        debug=False,
        num_devices=NCORES,
    )

    qhw = nc.dram_tensor("qhw", [BPC * HW, D], F32, kind="ExternalInput")
    auxS = nc.dram_tensor("auxS", [128, AUXS_W], F32, kind="ExternalInput")
    auxL = nc.dram_tensor("auxL", [128, AUXL_W], F32, kind="ExternalInput")
    out = nc.dram_tensor("out", [BPC, D], F32, kind="ExternalOutput")

    with tile.TileContext(nc) as tc:
        with (
            tc.tile_pool(name="sb", bufs=1) as sp,
            tc.tile_pool(name="ps", bufs=1, space="PSUM") as pp,
        ):
            # ---- input DMAs: small critical constants first ---------------
            aS = sp.tile([128, AUXS_W], F32)
            nc.sync.dma_start(out=aS[:], in_=auxS[:])
            aL = sp.tile([128, AUXL_W], F32)
            nc.sync.dma_start(out=aL[:], in_=auxL[:])

            ct0 = aS[:, 0:8]
            ct1 = aS[:, 8:16]
            wp0 = aS[:, 16:18]
            wp1 = aS[:, 18:20]
            selc = aS[0:2, 20:148]
            oi = aS[:, 148:149]
            boffc = aS[0:64, 153:161]   # [64, 8], col b = b*HW
            wa0 = aL[:, 0:128]
            wa1 = aL[:, 128:256]
            ident = aL[:, 256:384]
            band1 = aL[:, 384:512]
            e0sel = aL[:, 512:640]
            constm = aL[:, 640:672]     # only partition 0 is read (e0sel)

            # ---- early memset (no data deps) ------------------------------
            qgall = sp.tile([128, NT * QGS], F32R)  # gather tiles + ones cols
            nc.gpsimd.memset(qgall[:].bitcast(F32), 1.0)

            # ---- p_t: ptT[2,8] = (W_p c_t)^T, broadcast 128*sigmoid -------
            ptT_ps = pp.tile([2, BPC], F32)
            nc.tensor.matmul(out=ptT_ps[:], lhsT=wp0, rhs=ct0, start=True, stop=False)
            nc.tensor.matmul(out=ptT_ps[:], lhsT=wp1, rhs=ct1, start=False, stop=True)
            sig8 = sp.tile([2, BPC], F32)
            nc.scalar.activation(out=sig8[:], in_=ptT_ps[:], func=ACT.Sigmoid)
            # selc rows carry 128.0 -> ptb = 128*sigmoid; row part in
            # partitions 0:64 (p_t[:,0]), col part in partitions 64:128
            ptb_ps = pp.tile([128, BPC], F32)
            nc.tensor.matmul(out=ptb_ps[:], lhsT=selc, rhs=sig8[:], start=True, stop=True)

            # ---- round via f32->i32->f32 cast -----------------------------
            pri = sp.tile([128, BPC], I32)
            nc.vector.tensor_copy(pri[:], ptb_ps[:])
            prf = sp.tile([128, BPC], F32)
            nc.vector.tensor_copy(prf[:], pri[:])

            # ---- window indices: a=max(p+o,0); rr=a*(a<129) ---------------
            aa = sp.tile([128, BPC], F32)
            nc.vector.tensor_scalar(
                out=aa[:], in0=prf[:], scalar1=oi, scalar2=0.0,
                op0=AOP.add, op1=AOP.max,
            )
            amask = sp.tile([128, BPC], F32)
            nc.vector.tensor_scalar(
                out=amask[:], in0=aa[:], scalar1=float(ROWS), scalar2=None, op0=AOP.is_lt
            )
            rr = sp.tile([128, BPC], F32)
            nc.vector.tensor_tensor(out=rr[:], in0=aa[:], in1=amask[:], op=AOP.mult)
            # rm1 = max(rr-1, 0)
            rm1 = sp.tile([128, BPC], F32)
            nc.vector.tensor_scalar(
                out=rm1[:], in0=rr[:], scalar1=1.0, scalar2=0.0,
                op0=AOP.subtract, op1=AOP.max,
            )
            # flat row index gidx = rm1_r*W + rm1_c + b*HW  (in [64, 8])
            cm = sp.tile([64, BPC], F32)
            nc.vector.tensor_copy(cm[:], rm1[64:128, :])       # c-half to base 0
            rm1x = sp.tile([64, BPC], F32)
            nc.vector.tensor_scalar_mul(rm1x[:], rm1[0:64, :], float(W))
            gidx = sp.tile([64, BPC], F32)
            nc.vector.tensor_tensor(out=gidx[:], in0=rm1x[:], in1=cm[:], op=AOP.add)
            gb = sp.tile([64, BPC], F32)
            nc.vector.tensor_tensor(out=gb[:], in0=gidx[:], in1=boffc, op=AOP.add)
            # interleave batch pairs into gather-tile columns [128, 4]
            idxf = sp.tile([128, NT], F32)
            nc.vector.tensor_copy(idxf[0:64, :], gb[:, 0:8:2])
            nc.vector.tensor_copy(idxf[64:128, :], gb[:, 1:8:2])
            idx128 = sp.tile([128, NT], I32)
            nc.vector.tensor_copy(idx128[:], idxf[:])

            # ---- sparse gather: 512 rows x 512B ---------------------------
            qgv = qgall[:].rearrange("p (t x) -> p t x", x=QGS)
            if GATHER_SINGLE:
                nc.gpsimd.indirect_dma_start(
                    out=qgv[:, :, 0:D],
                    out_offset=None,
                    in_=qhw[:].bitcast(F32R),
                    in_offset=IndirectOffsetOnAxis(ap=idx128[:, 0:NT], axis=0),
                )
            else:
                for t in range(NT):
                    nc.gpsimd.indirect_dma_start(
                        out=qgall[:, t * QGS : t * QGS + D],
                        out_offset=None,
                        in_=qhw[:].bitcast(F32R),
                        in_offset=IndirectOffsetOnAxis(ap=idx128[:, t : t + 1], axis=0),
                    )

            # ---- shift/valid pre-terms (overlap the gather) ---------------
            # mask uses the linear form -BIG*(2 - vr - vc) with power-of-two
            # constants so the valid-lane cancellation -2^100 + 2^99+2^99 is
            # EXACT in f32; the small -shift term is only added afterwards.
            rexpd = sp.tile([128, BPC], F32)
            nc.vector.tensor_tensor(out=rexpd[:], in0=rm1[:], in1=ptb_ps[:], op=AOP.subtract)
            rpos = sp.tile([128, BPC], F32)
            nc.vector.tensor_scalar(
                out=rpos[:], in0=rr[:], scalar1=0.0, scalar2=None, op0=AOP.is_gt
            )
            pre1 = sp.tile([128, BPC], F32)
            nc.vector.tensor_tensor(out=pre1[:], in0=rexpd[:], in1=rexpd[:], op=AOP.mult)
            maskt = sp.tile([128, BPC], F32)
            nc.vector.tensor_scalar(
                out=maskt[:], in0=rpos[:], scalar1=float(2.0**99), scalar2=None,
                op0=AOP.mult,
            )
            shiftt = sp.tile([128, BPC], F32)
            nc.vector.tensor_scalar(
                out=shiftt[:], in0=pre1[:], scalar1=-0.125, scalar2=None, op0=AOP.mult
            )

            # ---- vT[d,b] = sum_c W_a[c,d] c_t[c,b] ------------------------
            vT_ps = pp.tile([D, BPC], F32)
            nc.tensor.matmul(out=vT_ps[:], lhsT=wa0, rhs=ct0, start=True, stop=False)
            nc.tensor.matmul(out=vT_ps[:], lhsT=wa1, rhs=ct1, start=False, stop=True)
            vT_sb = sp.tile([D, BPC], F32R)
            nc.vector.tensor_copy(vT_sb[:], vT_ps[:])

            # ---- scores PSUM: col consts, mask band, then shift band ------
            # order matters: the +-2^100 cancellation must complete before
            # the small -shift/score terms are accumulated
            scores_ps = pp.tile([128, NT * BPC], F32)
            nc.tensor.matmul(
                out=scores_ps[:], lhsT=e0sel, rhs=constm,
                start=True, stop=False, skip_group_check=True,
            )
            nc.tensor.matmul(
                out=scores_ps[:], lhsT=band1,
                rhs=maskt[:].unsqueeze(1).to_broadcast([128, NT, BPC]),
                start=False, stop=False, skip_group_check=True,
            )
            nc.tensor.matmul(
                out=scores_ps[:], lhsT=band1,
                rhs=shiftt[:].unsqueeze(1).to_broadcast([128, NT, BPC]),
                start=False, stop=False, skip_group_check=True,
            )

            # ---- transpose gathered rows, scores in [128k, 8b] strips -----
            qgT_sb = sp.tile([D, NT * 128], F32R)
            for t in range(NT):
                tr_ps = pp.tile([D, 128], F32, tag=f"tr{t % 2}")
                nc.tensor.transpose(tr_ps[:], qgall[:, t * QGS : t * QGS + D].bitcast(F32), ident)
                nc.vector.tensor_copy(qgT_sb[:, t * 128 : (t + 1) * 128], tr_ps[:])
                nc.tensor.matmul(
                    out=scores_ps[:, t * BPC : (t + 1) * BPC],
                    lhsT=qgT_sb[:, t * 128 : (t + 1) * 128],
                    rhs=vT_sb[:],
                    start=False, stop=(t == NT - 1), skip_group_check=True,
                )

            # ---- exp straight out of PSUM ---------------------------------
            e_sb = sp.tile([128, NT * BPC], F32R)
            nc.scalar.activation(out=e_sb[:], in_=scores_ps[:], func=ACT.Exp)

            # ---- unnormalized weighted sum + denominator in one PSUM ------
            # out_ps[b, 0:128] = sum_k e[k,b] qg[k,:], out_ps[b,128] = sum_k e[k,b]
            out_ps = pp.tile([BPC, D + 2], F32)
            for t in range(NT):
                nc.tensor.matmul(
                    out=out_ps[:],
                    lhsT=e_sb[:, t * BPC : (t + 1) * BPC],
                    rhs=qgall[:, t * QGS : t * QGS + D + 2],
                    start=(t == 0), stop=(t == NT - 1),
                )
            sinv = sp.tile([BPC, 1], F32)
            nc.vector.reciprocal(sinv[:], out_ps[:, D : D + 1])
            outf = sp.tile([BPC, D], F32)
            nc.vector.tensor_scalar(
                out=outf[:], in0=out_ps[:, 0:D], scalar1=sinv[:, 0:1], scalar2=None,
                op0=AOP.mult,
            )
            nc.sync.dma_start(out=out[:], in_=outf[:])

    nc.compile()
    return nc


_CACHE = {}


def _prep_in_maps(q, c_t, W_a, W_p):
    offs = (np.arange(R) - (R // 2 - 1)).astype(np.float32)  # [-3..4]
    p = np.arange(128)
    oi_np = np.where(p < 64, offs[(p // 8) % 8], offs[p % 8]).astype(np.float32)

    selc_np = np.zeros((2, 128), np.float32)
    selc_np[0, 0:64] = float(H)
    selc_np[1, 64:128] = float(H)

    auxS_base = np.zeros((128, AUXS_W), np.float32)
    auxS_base[:, 16:18] = W_p.T.astype(np.float32)[0:128]
    auxS_base[:, 18:20] = W_p.T.astype(np.float32)[128:256]
    auxS_base[0:2, 20:148] = selc_np
    auxS_base[:, 148] = oi_np
    auxS_base[0:64, 153:161] = (np.arange(BPC, dtype=np.float32) * HW)[None, :]

    # band1[q, p] = (q == p%64) + (q == 64 + p%64): sums r/c halves
    band1_np = np.zeros((128, 128), np.float32)
    for pp_ in range(128):
        band1_np[pp_ % 64, pp_] = 1.0
        band1_np[64 + pp_ % 64, pp_] = 1.0
    e0sel_np = np.zeros((128, 128), np.float32)
    e0sel_np[0, 0:64] = 1.0       # row 0 -> upper half (even batch real)
    e0sel_np[1, 64:128] = 1.0     # row 1 -> lower half (odd batch real)
    constm_np = np.full((2, NT * BPC), -(2.0**101), np.float32)
    for t in range(NT):
        constm_np[0, 8 * t + 2 * t] = -(2.0**100)
        constm_np[1, 8 * t + 2 * t + 1] = -(2.0**100)

    auxL_np = np.zeros((128, AUXL_W), np.float32)
    auxL_np[:, 0:128] = W_a.astype(np.float32)[0:128]
    auxL_np[:, 128:256] = W_a.astype(np.float32)[128:256]
    auxL_np[:, 256:384] = np.eye(128, dtype=np.float32)
    auxL_np[:, 384:512] = band1_np
    auxL_np[:, 512:640] = e0sel_np
    auxL_np[0:2, 640:672] = constm_np

    in_maps = []
    for c in range(NCORES):
        qs = q[c * BPC : (c + 1) * BPC]  # [BPC, D, H, W]
        qhw_np = np.ascontiguousarray(qs.transpose(0, 2, 3, 1)).reshape(BPC * HW, D)
        ctT_np = np.ascontiguousarray(c_t[c * BPC : (c + 1) * BPC].T)  # [CSZ, BPC]
        auxS_np = auxS_base.copy()
        auxS_np[:, 0:8] = ctT_np[0:128]
        auxS_np[:, 8:16] = ctT_np[128:256]
        in_maps.append({"qhw": qhw_np, "auxS": auxS_np, "auxL": auxL_np})
    return in_maps


def run(trace=False, tmpdir=None, **inputs):
    q = np.asarray(inputs["q"], dtype=np.float32)
    c_t = np.asarray(inputs["c_t"], dtype=np.float32)
    W_a = np.asarray(inputs["W_a"], dtype=np.float32)
    W_p = np.asarray(inputs["W_p"], dtype=np.float32)
    if "nc" not in _CACHE:
        _CACHE["nc"] = _build()
    in_maps = _prep_in_maps(q, c_t, W_a, W_p)
    res = _bu.run_bass_kernel_spmd(
        _CACHE["nc"], in_maps, core_ids=list(range(NCORES)), trace=trace,
        tmpdir=tmpdir,
    )
    outp = np.concatenate([r["out"] for r in res.results], axis=0)
    return outp, res


def kernel(**inputs):
    outp, _ = run(trace=False, **inputs)
    return outp
